# revision 1
# baseline (speedup 1.0000x reference)
"""Causal self-attention (dense transformer block) on 8 Trainium2 NeuronCores.

Sharding: tensor-parallel over heads x data-parallel over batch.
Core i handles batch i//4 and heads 4*(i%4) .. 4*(i%4)+3. Each core:
  1. qkv^T = (w_shard^T @ x^T + b)        -- PE, f32r
  2. V token-major via PE transpose (+ ones column for the softmax denom)
  3. per head: S^T[j,i] = k^T.T @ q^T (pre-scaled), exp on ACT, causal mask,
     o_unnorm^T = [V|1]^T @ P^T accumulated over j-tiles (row 64 = denom)
  4. normalize via ln/exp reciprocal + K=1 broadcast matmul + DVE multiply
  5. partial_out = o_norm^T.T @ w_proj_shard  (host sums partials + b_proj)

Work is emitted per head-pair so head-pair-1 QKV matmuls fill the PE while
the ACT engine runs head-pair-0's softmax exps. All matmuls use float32r
(full PE rate, ~1e-3 relative precision).
"""
import os
import sys

for _p in ("/opt/trn_rl_repo", "/root/.axon_site/_ro/trn_rl_repo"):
    if os.path.isdir(_p) and _p not in sys.path:
        sys.path.insert(0, _p)

import numpy as np
import concourse.bass as bass
import concourse.tile as tile
from concourse import mybir
from concourse.bass_utils import run_bass_kernel_spmd

F32R = mybir.dt.float32r
F32 = mybir.dt.float32
AF = mybir.ActivationFunctionType
MUL = mybir.AluOpType.mult

B, T, C = 2, 2048, 1024
HEADS, D = 16, 64
NCORES = 8
HPC = 4  # heads per core
SCALE = 1.0 / np.sqrt(np.float32(D))

MAX_WAITS = 1


def split_multi_waits(nc, max_waits=MAX_WAITS):
    """This walrus build rejects instructions carrying more than one sync-wait
    command. Split extras onto preceding same-engine NoOps (engine programs
    execute in order, so the gating is equivalent)."""
    idx = 0
    for f in nc.m.functions:
        for bb in f.blocks:
            out = []
            changed = False
            for inst in bb.instructions:
                si = inst.sync_info
                waits = list(si.on_wait) if si and si.on_wait else []
                if len(waits) > max_waits:
                    changed = True
                    for w in waits[:-max_waits]:
                        nop = mybir.InstNoOp(name=f"I-waitsplit-{idx}", ins=[], outs=[])
                        idx += 1
                        nop.engine = inst.engine
                        nop.sync_info = mybir.SyncInfo(on_wait=[w], on_update=[])
                        out.append(nop)
                    inst.sync_info = mybir.SyncInfo(
                        on_wait=waits[-max_waits:], on_update=list(si.on_update or [])
                    )
                out.append(inst)
            if changed:
                bb.instructions = out
    return nc


def build_nc(repeat=1, split=True):
    nc = bass.Bass(trn_type="TRN2")

    xT_d = nc.dram_tensor("xT", [C, T], F32R, kind="ExternalInput")
    wqkv_d = nc.dram_tensor("wqkv", [6, 8, 128, 128], F32R, kind="ExternalInput")
    bqkv_d = nc.dram_tensor("bqkv", [128, 6], F32, kind="ExternalInput")
    wproj_d = nc.dram_tensor("wproj", [2 * 128, C], F32R, kind="ExternalInput")
    mask_d = nc.dram_tensor("mask", [128, 128], F32R, kind="ExternalInput")
    ident_d = nc.dram_tensor("ident", [128, 128], F32R, kind="ExternalInput")
    onesb_d = nc.dram_tensor("onesb", [128, 64], F32R, kind="ExternalInput")
    onesv_d = nc.dram_tensor("onesv", [128, 16, 1], F32R, kind="ExternalInput")
    zeros_d = nc.dram_tensor("zeros", [128, 384], F32R, kind="ExternalInput")
    out_d = nc.dram_tensor("out", [T, C], F32, kind="ExternalOutput")

    rings = [nc.sync, nc.scalar]
    ring_i = [0]

    def ring():
        ring_i[0] += 1
        return rings[ring_i[0] % len(rings)]

    with tile.TileContext(nc) as tc:
        with tc.tile_pool(name="perm", bufs=1) as perm, \
             tc.tile_pool(name="xt", bufs=3) as xtp, \
             tc.tile_pool(name="pt", bufs=3) as ptp, \
             tc.tile_pool(name="otun", bufs=3) as otunp, \
             tc.tile_pool(name="otmp", bufs=2) as otmpp, \
             tc.tile_pool(name="stg", bufs=2) as stgp, \
             tc.tile_pool(name="psum", bufs=1, space="PSUM") as psp:
            qkvT = [perm.tile([128, T], F32R, tag=f"qkvT{i}", name=f"qkvT{i}")
                    for i in range(6)]
            wq = [perm.tile([128, 8, 128], F32R, tag=f"wq{blk}", name=f"wq{blk}")
                  for blk in range(6)]
            wproj_sb = [perm.tile([128, C], F32R, tag=f"wproj{i}", name=f"wproj{i}")
                        for i in range(2)]
            bias_sb = perm.tile([128, 6], F32, tag="bias", name="bias")
            mask_sb = perm.tile([128, 128], F32R, tag="mask", name="mask_sb")
            ident_sb = perm.tile([128, 128], F32R, tag="ident", name="ident_sb")
            onesb_sb = perm.tile([128, 64], F32R, tag="onesb", name="onesb_sb")
            zeros_sb = perm.tile([128, 384], F32R, tag="zeros", name="zeros_sb")
            vext = [perm.tile([128, 16, 65], F32R, tag=f"vext{h}", name=f"vext{h}")
                    for h in range(HPC)]
            onorm = [perm.tile([128, T], F32R, tag=f"onorm{i}", name=f"onorm{i}")
                     for i in range(2)]

            first_xts = []

            def load_qkv_chunks(tchs):
                """DMA x^T tiles and run the 6 qkv blocks for given t-chunks."""
                for tch in tchs:
                    if tch == 0 and first_xts:
                        xts = first_xts
                    else:
                        xts = []
                        for ct in range(8):
                            xt_t = xtp.tile([128, 512], F32R, tag=f"xt{ct}",
                                            name=f"xt{ct}")
                            ring().dma_start(
                                out=xt_t[:],
                                in_=xT_d[ct * 128 : (ct + 1) * 128,
                                         tch * 512 : (tch + 1) * 512],
                            )
                            xts.append(xt_t)
                    for blk in (0, 2, 4, 1, 3, 5):
                        ps = psp.tile([128, 512], F32, tag="qkv", bufs=2, name="ps_qkv")
                        for ct in range(8):
                            nc.tensor.matmul(
                                ps[:],
                                wq[blk][:, ct, :],
                                xts[ct][:],
                                start=(ct == 0),
                                stop=(ct == 7),
                            )
                        nc.vector.tensor_scalar_add(
                            out=qkvT[blk][:, tch * 512 : (tch + 1) * 512],
                            in0=ps[:],
                            scalar1=bias_sb[:, blk : blk + 1],
                        )

            def v_transposes(tjs, hps=(0, 1)):
                for hp in hps:
                    vb = qkvT[4 + hp]
                    for tj in tjs:
                        ps = psp.tile([128, 128], F32R, tag="qkv", bufs=2, name="ps_v")
                        nc.tensor.transpose(
                            ps[:], vb[:, tj * 128 : (tj + 1) * 128], ident_sb[:]
                        )
                        for sub in range(2):
                            h = 2 * hp + sub
                            nc.vector.tensor_copy(
                                out=vext[h][:, tj, 0:64],
                                in_=ps[:, sub * 64 : (sub + 1) * 64],
                            )

            def attention_half(h, ihalf):
                hp, sub = h // 2, h % 2
                r0 = 64 * sub
                qb, kb = qkvT[hp], qkvT[2 + hp]
                istart = ihalf * 1024
                ntj = 8 if ihalf == 0 else 16
                otun = otunp.tile([65, 1024], F32R, tag="otun", name="otun")
                ots = [psp.tile([65, 512], F32, tag=f"ot{t}", bufs=1, name=f"ot{t}")
                       for t in range(2)]
                for tj in range(ntj):
                    j0 = tj * 128
                    tis = [t for t in range(2) if istart + t * 512 + 511 >= j0]
                    st = psp.tile([128, 1024], F32, tag="st", bufs=2, name="st")
                    for t in tis:
                        nc.tensor.matmul(
                            st[:, t * 512 : (t + 1) * 512],
                            kb[r0 : r0 + 64, j0 : j0 + 128],
                            qb[r0 : r0 + 64,
                               istart + t * 512 : istart + (t + 1) * 512],
                            start=True,
                            stop=True,
                        )
                    ptt = ptp.tile([128, 1024], F32R, tag="pt", name="ptt")
                    lo = max(j0 - istart, 0)
                    pre = tis[0] * 512
                    if lo > pre:
                        nc.gpsimd.tensor_copy(
                            out=ptt[:, pre:lo], in_=zeros_sb[:, 0 : lo - pre]
                        )
                    nc.scalar.activation(
                        out=ptt[:, lo:1024], in_=st[:, lo:1024], func=AF.Exp
                    )
                    diag = (j0 >= istart)
                    if diag:
                        nc.vector.tensor_tensor(
                            out=ptt[:, lo : lo + 128],
                            in0=ptt[:, lo : lo + 128],
                            in1=mask_sb[:],
                            op=MUL,
                        )
                    # non-diagonal PV first: the masked sub-block gates only
                    # the diagonal i-tile
                    dt_ = (j0 - istart) // 512 if diag else -1
                    for t in sorted(tis, key=lambda t: t == dt_):
                        gti = ihalf * 2 + t
                        last_tj = min(4 * gti + 3, ntj - 1)
                        nc.tensor.matmul(
                            ots[t][:],
                            vext[h][:, tj, :],
                            ptt[:, t * 512 : (t + 1) * 512],
                            start=(tj == 0),
                            stop=(tj == last_tj),
                        )
                    for t in tis:
                        gti = ihalf * 2 + t
                        if tj == min(4 * gti + 3, ntj - 1):
                            sl = slice(t * 512, (t + 1) * 512)
                            nc.vector.tensor_copy(out=otun[:, sl], in_=ots[t][:])
                            # finalize this i-tile now (causal: it's complete)
                            nc.scalar.activation(out=otun[64:65, sl],
                                                 in_=otun[64:65, sl], func=AF.Ln)
                            nc.scalar.activation(out=otun[64:65, sl],
                                                 in_=otun[64:65, sl],
                                                 func=AF.Exp, scale=-1.0)
                            bc = psp.tile([64, 512], F32, tag=f"ot{t}", bufs=1,
                                          name="bc")
                            nc.tensor.matmul(bc[:], onesb_sb[64:65, :],
                                             otun[64:65, sl],
                                             start=True, stop=True)
                            goff = istart + t * 512
                            if sub == 0:
                                dst = onorm[hp][0:64, goff : goff + 512]
                            else:
                                ot_s = otmpp.tile([64, 512], F32R, tag="otmp",
                                                  name="ot_s")
                                dst = ot_s[0:64, :]
                            nc.vector.tensor_tensor(
                                out=dst[:], in0=otun[0:64, sl], in1=bc[:], op=MUL
                            )
                            if sub == 1:
                                nc.gpsimd.dma_start(
                                    out=onorm[hp][64:128, goff : goff + 512],
                                    in_=ot_s[0:64, :],
                                )

            out_rings = [nc.sync, nc.scalar]

            def proj_half(tbs):
                for tb in tbs:
                    sg = stgp.tile([128, 1024], F32, tag="sg", name="sg")
                    for cc in range(2):
                        ps = psp.tile([128, 512], F32, tag="qkv", bufs=2, name="ps_j")
                        for hp2 in range(2):
                            nc.tensor.matmul(
                                ps[:],
                                onorm[hp2][:, tb * 128 : (tb + 1) * 128],
                                wproj_sb[hp2][:, cc * 512 : (cc + 1) * 512],
                                start=(hp2 == 0),
                                stop=(hp2 == 1),
                            )
                        eng = nc.vector if cc == 0 else nc.scalar
                        if cc == 0:
                            nc.vector.tensor_copy(
                                out=sg[:, 0:512], in_=ps[:]
                            )
                        else:
                            nc.scalar.activation(
                                out=sg[:, 512:1024], in_=ps[:], func=AF.Copy
                            )
                    out_rings[tb % 2].dma_start(
                        out=out_d[tb * 128 : (tb + 1) * 128, :], in_=sg[:]
                    )

            # ---- emission order: interleave DMA-heavy, PE-heavy, ACT-heavy ----
            for _rep in range(repeat):
              del first_xts[:]
              nc.sync.dma_start(out=bias_sb[:], in_=bqkv_d[:])
              wq_order = (0, 2, 4, 1, 3, 5)
              for blk0 in (0, 2):
                  for h8 in (0, 1):
                      ring().dma_start(
                          out=wq[blk0][:, 4 * h8 : 4 * h8 + 4, :],
                          in_=wqkv_d[blk0, 4 * h8 : 4 * h8 + 4].rearrange(
                              "ct p col -> p ct col"
                          ),
                      )
              for ct in range(8):
                  xt_t = xtp.tile([128, 512], F32R, tag=f"xt{ct}", name=f"xt{ct}")
                  ring().dma_start(
                      out=xt_t[:], in_=xT_d[ct * 128 : (ct + 1) * 128, 0:512]
                  )
                  first_xts.append(xt_t)
              for blk in (4, 1, 3, 5):
                  ring().dma_start(
                      out=wq[blk][:], in_=wqkv_d[blk].rearrange("ct p col -> p ct col")
                  )
              load_qkv_chunks([0, 1])
              nc.scalar.dma_start(out=mask_sb[:], in_=mask_d[:])
              nc.sync.dma_start(out=ident_sb[:], in_=ident_d[:])
              nc.scalar.dma_start(out=onesb_sb[:], in_=onesb_d[:])
              nc.scalar.dma_start(out=zeros_sb[:], in_=zeros_d[:])
              for h in range(HPC):
                  nc.sync.dma_start(out=vext[h][:, :, 64:65], in_=onesv_d[:])
              v_transposes(range(8), hps=(0,))
              attention_half(0, 0)
              attention_half(1, 0)
              v_transposes(range(8), hps=(1,))
              load_qkv_chunks([2, 3])
              attention_half(2, 0)
              attention_half(3, 0)
              v_transposes(range(8, 16), hps=(0,))
              for i in range(2):
                  ring().dma_start(
                      out=wproj_sb[i][:], in_=wproj_d[i * 128 : (i + 1) * 128, :]
                  )
              proj_half(range(8))
              attention_half(0, 1)
              attention_half(1, 1)
              v_transposes(range(8, 16), hps=(1,))
              attention_half(2, 1)
              attention_half(3, 1)
              proj_half(range(8, 16))
    return split_multi_waits(nc) if split else nc


def host_shards(x, w_attn, b_attn, w_proj):
    """Per-core input maps. Pure layout/scalar transforms of the inputs."""
    x = np.asarray(x, dtype=np.float32)
    w_attn = np.asarray(w_attn, dtype=np.float32)
    b_attn = np.asarray(b_attn, dtype=np.float32)
    w_proj = np.asarray(w_proj, dtype=np.float32)

    mask = np.triu(np.ones((128, 128), np.float32))  # mask[j,i] = 1 iff j <= i
    ident = np.eye(128, dtype=np.float32)
    onesb = np.ones((128, 64), np.float32)
    onesv = np.ones((128, 16, 1), np.float32)
    zeros = np.zeros((128, 384), np.float32)

    in_maps = []
    for core in range(NCORES):
        b = core // 4
        heads = [4 * (core % 4) + k for k in range(HPC)]
        # qkvT block layout: blk j in 0..5 = [q-hp0, q-hp1, k-hp0, k-hp1, v-hp0, v-hp1]
        wqkv = np.zeros((6, 8, 128, 128), np.float32)
        bq = np.zeros((128, 6), np.float32)
        for j, (base, scl) in enumerate(((0, SCALE), (C, 1.0), (2 * C, 1.0))):
            for hp in range(2):
                blk = 2 * j + hp
                for s, h in enumerate((heads[2 * hp], heads[2 * hp + 1])):
                    cols = w_attn[:, base + h * D : base + (h + 1) * D] * scl
                    for ct in range(8):
                        wqkv[blk, ct, :, s * 64 : (s + 1) * 64] = (
                            cols[ct * 128 : (ct + 1) * 128, :]
                        )
                    bq[s * 64 : (s + 1) * 64, blk] = (
                        b_attn[base + h * D : base + (h + 1) * D] * scl
                    )
        wproj_s = np.ascontiguousarray(
            np.concatenate([w_proj[h * D : (h + 1) * D, :] for h in heads], axis=0)
        )  # [256, 1024]
        xT = np.ascontiguousarray(x[b].T)  # [1024, 2048]
        in_maps.append(
            {
                "xT": xT,
                "wqkv": wqkv,
                "bqkv": bq,
                "wproj": wproj_s,
                "mask": mask,
                "ident": ident,
                "onesb": onesb,
                "onesv": onesv,
                "zeros": zeros,
            }
        )
    return in_maps


_NC = None


def _get_nc():
    global _NC
    if _NC is None:
        _NC = build_nc()
    return _NC


def kernel(x, w_attn, b_attn, w_proj, b_proj):
    in_maps = host_shards(x, w_attn, b_attn, w_proj)
    nc = _get_nc()
    res = run_bass_kernel_spmd(nc, in_maps, core_ids=list(range(NCORES)))
    out = np.zeros((B, T, C), np.float64)
    for core in range(NCORES):
        out[core // 4] += res.results[core]["out"].astype(np.float64)
    out += np.asarray(b_proj, dtype=np.float64)[None, None, :]
    return out.astype(np.float32)



# revision 32
# speedup vs baseline: 1.2088x; 1.2088x over previous
"""Causal self-attention (dense transformer block) on 8 Trainium2 NeuronCores.

Sharding: tensor-parallel over heads x data-parallel over batch.
Core i handles batch i//4 and heads 4*(i%4) .. 4*(i%4)+3. Each core:
  1. qkv^T = (w_shard^T @ x^T + b)        -- PE, bf16 in / f32 psum
  2. V token-major via PE transpose (+ ones column for the softmax denom)
  3. per head: S^T[j,i] = k^T.T @ q^T (pre-scaled), exp on ACT -> bf16 P,
     causal mask, o_unnorm^T = [V|1]^T @ P^T accumulated over j-tiles
     (row 64 = denom); S/PV matmuls are trimmed to the causal column range
  4. normalize via DVE reciprocal + K=1 broadcast matmul + DVE multiply
  5. partial_out = o_norm^T.T @ w_proj_shard  (host sums partials + b_proj)

Work is emitted per head-pair so head-pair-1 QKV matmuls fill the PE while
the ACT engine runs head-pair-0's softmax exps. q/k/v/P/weights are bf16
(PE full rate incl. <256-wide tiles, half DMA traffic, 2-4x DVE); scores,
PV accumulation and the softmax denominator stay f32.
"""
import os
import sys

for _p in ("/opt/trn_rl_repo", "/root/.axon_site/_ro/trn_rl_repo"):
    if os.path.isdir(_p) and _p not in sys.path:
        sys.path.insert(0, _p)

import numpy as np
import concourse.bass as bass
import concourse.tile as tile
from concourse import mybir
from concourse.bass_utils import run_bass_kernel_spmd

F32R = mybir.dt.float32r
F32 = mybir.dt.float32
BF16 = mybir.dt.bfloat16
BF16_NP = mybir.dt.np(BF16)
F16 = mybir.dt.float16
AF = mybir.ActivationFunctionType
MUL = mybir.AluOpType.mult

B, T, C = 2, 2048, 1024
HEADS, D = 16, 64
NCORES = 8
HPC = 4  # heads per core
SCALE = 1.0 / np.sqrt(np.float32(D))

MAX_WAITS = 1


def split_multi_waits(nc, max_waits=MAX_WAITS):
    """This walrus build rejects instructions carrying more than one sync-wait
    command. Split extras onto preceding same-engine NoOps (engine programs
    execute in order, so the gating is equivalent)."""
    idx = 0
    for f in nc.m.functions:
        for bb in f.blocks:
            out = []
            changed = False
            for inst in bb.instructions:
                si = inst.sync_info
                waits = list(si.on_wait) if si and si.on_wait else []
                if len(waits) > max_waits:
                    changed = True
                    for w in waits[:-max_waits]:
                        nop = mybir.InstNoOp(name=f"I-waitsplit-{idx}", ins=[], outs=[])
                        idx += 1
                        nop.engine = inst.engine
                        nop.sync_info = mybir.SyncInfo(on_wait=[w], on_update=[])
                        out.append(nop)
                    inst.sync_info = mybir.SyncInfo(
                        on_wait=waits[-max_waits:], on_update=list(si.on_update or [])
                    )
                out.append(inst)
            if changed:
                bb.instructions = out
    return nc


def build_nc(repeat=1, split=True):
    nc = bass.Bass(trn_type="TRN2")

    xT_d = nc.dram_tensor("xT", [C, T], BF16, kind="ExternalInput")
    # p-major weight layout: per block [128 part, 8 ct, 128 col] so each
    # partition row is one contiguous 2KB DMA descriptor (no rearrange)
    wqkv_d = nc.dram_tensor("wqkv", [6, 128, 8, 128], BF16, kind="ExternalInput")
    bqkv_d = nc.dram_tensor("bqkv", [128, 6], F32, kind="ExternalInput")
    wproj_d = nc.dram_tensor("wproj", [2 * 128, C], BF16, kind="ExternalInput")
    mask_d = nc.dram_tensor("mask", [128, 128], BF16, kind="ExternalInput")
    ident_d = nc.dram_tensor("ident", [128, 128], BF16, kind="ExternalInput")
    onesb_d = nc.dram_tensor("onesb", [128, 64], F32R, kind="ExternalInput")
    onesv_d = nc.dram_tensor("onesv", [128, 16, 1], BF16, kind="ExternalInput")
    out_d = nc.dram_tensor("out", [T, C], F16, kind="ExternalOutput")

    rings = [nc.sync, nc.scalar]
    ring_i = [0]

    def ring():
        ring_i[0] += 1
        return rings[ring_i[0] % len(rings)]

    with tile.TileContext(nc) as tc:
        with tc.tile_pool(name="perm", bufs=1) as perm, \
             tc.tile_pool(name="xt", bufs=3) as xtp, \
             tc.tile_pool(name="pt", bufs=3) as ptp, \
             tc.tile_pool(name="otun", bufs=3) as otunp, \
             tc.tile_pool(name="otmp", bufs=2) as otmpp, \
             tc.tile_pool(name="stg", bufs=2) as stgp, \
             tc.tile_pool(name="psum", bufs=1, space="PSUM") as psp:
            qkvT = [perm.tile([128, T], BF16, tag=f"qkvT{i}", name=f"qkvT{i}")
                    for i in range(6)]
            wq = [perm.tile([128, 8, 128], BF16, tag=f"wq{blk}", name=f"wq{blk}")
                  for blk in range(6)]
            wproj_sb = [perm.tile([128, C], BF16, tag=f"wproj{i}", name=f"wproj{i}")
                        for i in range(2)]
            bias_sb = perm.tile([128, 6], F32, tag="bias", name="bias")
            mask_sb = perm.tile([128, 128], BF16, tag="mask", name="mask_sb")
            ident_sb = perm.tile([128, 128], BF16, tag="ident", name="ident_sb")
            onesb_sb = perm.tile([128, 64], F32R, tag="onesb", name="onesb_sb")
            vext = [perm.tile([128, 16, 65], BF16, tag=f"vext{h}", name=f"vext{h}")
                    for h in range(HPC)]
            onorm = [perm.tile([128, T], BF16, tag=f"onorm{i}", name=f"onorm{i}")
                     for i in range(2)]

            chunk_xts = {}

            def issue_xt_dmas(tch):
                xts = []
                for ct in range(8):
                    xt_t = xtp.tile([128, 512], BF16, tag=f"xt{ct}",
                                    name=f"xt{ct}")
                    ring().dma_start(
                        out=xt_t[:],
                        in_=xT_d[ct * 128 : (ct + 1) * 128,
                                 tch * 512 : (tch + 1) * 512],
                    )
                    xts.append(xt_t)
                chunk_xts[tch] = xts

            def emit_qkv_block(tch, blk):
                xts = chunk_xts[tch]
                ps = psp.tile([128, 512], F32, tag="qkv", bufs=2, name="ps_qkv")
                for ct in range(8):
                    nc.tensor.matmul(
                        ps[:],
                        wq[blk][:, ct, :],
                        xts[ct][:],
                        start=(ct == 0),
                        stop=(ct == 7),
                    )
                nc.vector.tensor_scalar_add(
                    out=qkvT[blk][:, tch * 512 : (tch + 1) * 512],
                    in0=ps[:],
                    scalar1=bias_sb[:, blk : blk + 1],
                )

            def v_transposes(tjs, hps=(0, 1)):
                for hp in hps:
                    vb = qkvT[4 + hp]
                    for tj in tjs:
                        ps = psp.tile([128, 128], BF16, tag="qkv", bufs=2, name="ps_v")
                        nc.tensor.transpose(
                            ps[:], vb[:, tj * 128 : (tj + 1) * 128], ident_sb[:]
                        )
                        for sub in range(2):
                            h = 2 * hp + sub
                            nc.vector.tensor_copy(
                                out=vext[h][:, tj, 0:64],
                                in_=ps[:, sub * 64 : (sub + 1) * 64],
                            )

            def attention_half(h, ihalf, fillers=()):
                """fillers: ordered callables emitting independent PE work;
                one is popped after each tj's PV so the PE has fill-in while
                the ACT engine runs this half's exps."""
                fillers = list(fillers)
                hp, sub = h // 2, h % 2
                r0 = 64 * sub
                qb, kb = qkvT[hp], qkvT[2 + hp]
                istart = ihalf * 1024
                ntj = 8 if ihalf == 0 else 16
                otun = otunp.tile([65, 1024], F32R, tag="otun", name="otun")
                ots = [psp.tile([128, 512], F32, tag=f"ot{t}", bufs=1,
                                name=f"ot{t}")
                       for t in range(2)]

                def emit_S(tj):
                    j0 = tj * 128
                    lo = max(j0 - istart, 0)
                    tis = [t for t in range(2) if istart + t * 512 + 511 >= j0]
                    st = psp.tile([128, 1024], F32, tag="st", bufs=2, name="st")
                    for t in tis:
                        c0 = max(lo, t * 512)  # causal trim within the slab
                        nc.tensor.matmul(
                            st[:, c0 : (t + 1) * 512],
                            kb[r0 : r0 + 64, j0 : j0 + 128],
                            qb[r0 : r0 + 64,
                               istart + c0 : istart + (t + 1) * 512],
                            start=True,
                            stop=True,
                        )
                    return (tj, j0, lo, tis, st)

                pend = emit_S(0)
                for tj in range(ntj):
                    # emit S(tj+1) ahead of PV(tj): the PE runs it while the
                    # ACT engine computes exp(tj), instead of stalling on it
                    nxt = emit_S(tj + 1) if tj + 1 < ntj else None
                    _, j0, lo, tis, st = pend
                    ptt = ptp.tile([128, 1024], BF16, tag="pt", name="ptt")
                    diag = (j0 >= istart)
                    dt_ = (j0 - istart) // 512 if diag else -1
                    nc.scalar.activation(
                        out=ptt[:, lo:1024], in_=st[:, lo:1024], func=AF.Exp
                    )
                    if diag:
                        nc.gpsimd.tensor_tensor(
                            out=ptt[:, lo : lo + 128],
                            in0=ptt[:, lo : lo + 128],
                            in1=mask_sb[:],
                            op=MUL,
                        )
                    # non-diagonal PV first: the masked sub-block gates only
                    # the diagonal i-tile
                    for t in sorted(tis, key=lambda t: t == dt_):
                        c0 = max(lo, t * 512)
                        gti = ihalf * 2 + t
                        last_tj = min(4 * gti + 3, ntj - 1)
                        nc.tensor.matmul(
                            ots[t][0:65, c0 - t * 512 : 512],
                            vext[h][:, tj, :],
                            ptt[:, c0 : (t + 1) * 512],
                            start=(tj == 0),
                            stop=(tj == last_tj),
                        )
                    for t in tis:
                        gti = ihalf * 2 + t
                        if tj == min(4 * gti + 3, ntj - 1):
                            sl = slice(t * 512, (t + 1) * 512)
                            nc.vector.tensor_copy(out=otun[:, sl],
                                                  in_=ots[t][0:65, :])
                            # finalize this i-tile now (causal: it's complete)
                            with nc.allow_low_precision(reason="f32r is f32-width"):
                                nc.vector.reciprocal(out=otun[64:65, sl],
                                                     in_=otun[64:65, sl])
                            bc = psp.tile([64, 512], F32, tag=f"ot{t}", bufs=1,
                                          name="bc")
                            nc.tensor.matmul(bc[:], onesb_sb[64:65, :],
                                             otun[64:65, sl],
                                             start=True, stop=True)
                            goff = istart + t * 512
                            if sub == 0:
                                dst = onorm[hp][0:64, goff : goff + 512]
                            else:
                                ot_s = otmpp.tile([64, 512], BF16, tag="otmp",
                                                  name="ot_s")
                                dst = ot_s[0:64, :]
                            nc.vector.tensor_tensor(
                                out=dst[:], in0=otun[0:64, sl], in1=bc[:], op=MUL
                            )
                            if sub == 1:
                                # partition-shift rows 0:64 -> 64:128 on the
                                # PE (identity matmul into psum offset 64),
                                # then an aligned DVE copy into onorm
                                shf = psp.tile([128, 512], F32, tag=f"ot{t}",
                                               bufs=1, name="shf")
                                nc.tensor.matmul(shf[64:128, :],
                                                 ident_sb[0:64, 0:64],
                                                 ot_s[0:64, :],
                                                 start=True, stop=True)
                                nc.vector.tensor_copy(
                                    out=onorm[hp][64:128, goff : goff + 512],
                                    in_=shf[64:128, :],
                                )
                    if fillers:
                        fillers.pop(0)()
                    pend = nxt
                for f in fillers:
                    f()

            out_rings = [nc.sync, nc.scalar]

            def emit_proj_tile(tb, tags=("qkv", "qkv")):
                sg = stgp.tile([128, 1024], F16, tag="sg", bufs=4, name="sg")
                for cc in range(2):
                    ps = psp.tile([128, 512], F32, tag=tags[cc], bufs=2, name="ps_j")
                    for hp2 in range(2):
                        nc.tensor.matmul(
                            ps[:],
                            onorm[hp2][:, tb * 128 : (tb + 1) * 128],
                            wproj_sb[hp2][:, cc * 512 : (cc + 1) * 512],
                            start=(hp2 == 0),
                            stop=(hp2 == 1),
                        )
                    sl = slice(cc * 512, (cc + 1) * 512)
                    if cc == 0:
                        nc.vector.tensor_copy(out=sg[:, sl], in_=ps[:])
                    else:
                        nc.scalar.activation(out=sg[:, sl], in_=ps[:],
                                             func=AF.Copy)
                    # fly each 512-col half as soon as its copy lands
                    out_rings[(2 * tb + cc) % 2].dma_start(
                        out=out_d[tb * 128 : (tb + 1) * 128, sl],
                        in_=sg[:, sl],
                    )

            def proj_half(tbs):
                # the attention psum banks are idle in the tail: borrow the
                # st bank so proj pipelines across 4 psum buffers
                for tb in tbs:
                    emit_proj_tile(tb, tags=("st", "qkv"))

            def qkvf(tch, blk):
                return lambda: emit_qkv_block(tch, blk)

            def vtrf(tjs, hp):
                return lambda: v_transposes(tjs, hps=(hp,))

            def projf(tb):
                return lambda: emit_proj_tile(tb)

            # ---- emission order: hp0's qkv first so attention starts early;
            # remaining qkv blocks / V transposes / proj tiles are injected as
            # per-tj fillers into the attention halves so the PE never idles
            # while ACT runs the softmax exps.
            for _rep in range(repeat):
              chunk_xts.clear()
              # startup: first QKV block (blk 0) needs wq[0] + the xt tiles
              # in ct order; the sim's DMA engines are a serial resource, so
              # issue the blocking transfers (xt0/1, wq0) first and stream.
              xts0 = []
              nc.sync.dma_start(out=wq[0][:, 0:4, :], in_=wqkv_d[0, :, 0:4])
              nc.scalar.dma_start(out=wq[0][:, 4:8, :], in_=wqkv_d[0, :, 4:8])
              for ct in range(0, 8):
                  xt_t = xtp.tile([128, 512], BF16, tag=f"xt{ct}", name=f"xt{ct}")
                  ring().dma_start(
                      out=xt_t[:], in_=xT_d[ct * 128 : (ct + 1) * 128, 0:512]
                  )
                  xts0.append(xt_t)
              chunk_xts[0] = xts0
              nc.gpsimd.dma_start(out=bias_sb[:], in_=bqkv_d[:])
              for blk in (2, 4, 1, 3, 5):
                  ring().dma_start(out=wq[blk][:], in_=wqkv_d[blk])
              issue_xt_dmas(1)
              for blk in (0, 2, 4):
                  emit_qkv_block(0, blk)
              nc.scalar.dma_start(out=mask_sb[:], in_=mask_d[:])
              nc.sync.dma_start(out=ident_sb[:], in_=ident_d[:])
              nc.scalar.dma_start(out=onesb_sb[:], in_=onesb_d[:])
              for h in range(HPC):
                  nc.sync.dma_start(out=vext[h][:, :, 64:65], in_=onesv_d[:])
              for blk in (0, 2, 4):
                  emit_qkv_block(1, blk)
              v_transposes(range(8), hps=(0,))
              issue_xt_dmas(2)
              attention_half(0, 0, fillers=[
                  qkvf(0, 1), qkvf(0, 5), qkvf(1, 5), vtrf(range(0, 4), 1),
                  qkvf(1, 1), vtrf(range(4, 8), 1)])
              issue_xt_dmas(3)
              attention_half(1, 0, fillers=[
                  qkvf(0, 3), qkvf(1, 3), qkvf(2, 0)])
              attention_half(2, 0, fillers=[qkvf(2, 2), qkvf(2, 4)])
              for i in range(2):
                  ring().dma_start(
                      out=wproj_sb[i][:], in_=wproj_d[i * 128 : (i + 1) * 128, :]
                  )
              attention_half(3, 0, fillers=[qkvf(3, 0), qkvf(3, 2)])
              v_transposes(range(8, 12), hps=(0,))
              attention_half(0, 1, fillers=[
                  qkvf(3, 4), vtrf(range(12, 14), 0), vtrf(range(14, 16), 0),
                  qkvf(2, 1), projf(0), projf(1)])
              attention_half(1, 1, fillers=[
                  qkvf(2, 5), qkvf(3, 5), vtrf(range(8, 12), 1),
                  vtrf(range(12, 16), 1), qkvf(2, 3), qkvf(3, 1), projf(2)])
              attention_half(2, 1, fillers=[
                  qkvf(3, 3), projf(3), projf(4), projf(5)])
              attention_half(3, 1, fillers=[projf(6), projf(7)])
              proj_half(range(8, 16))
    return split_multi_waits(nc) if split else nc


def host_shards(x, w_attn, b_attn, w_proj):
    """Per-core input maps. Pure layout/scalar transforms of the inputs."""
    x = np.asarray(x, dtype=np.float32)
    w_attn = np.asarray(w_attn, dtype=np.float32)
    b_attn = np.asarray(b_attn, dtype=np.float32)
    w_proj = np.asarray(w_proj, dtype=np.float32)

    mask = np.triu(np.ones((128, 128), np.float32))  # mask[j,i] = 1 iff j <= i
    ident = np.eye(128, dtype=np.float32)
    onesb = np.ones((128, 64), np.float32)
    onesv = np.ones((128, 16, 1), np.float32)

    in_maps = []
    for core in range(NCORES):
        b = core // 4
        heads = [4 * (core % 4) + k for k in range(HPC)]
        # qkvT block layout: blk j in 0..5 = [q-hp0, q-hp1, k-hp0, k-hp1, v-hp0, v-hp1]
        # p-major: wqkv[blk, p, ct, col]
        wqkv = np.zeros((6, 128, 8, 128), np.float32)
        bq = np.zeros((128, 6), np.float32)
        for j, (base, scl) in enumerate(((0, SCALE), (C, 1.0), (2 * C, 1.0))):
            for hp in range(2):
                blk = 2 * j + hp
                for s, h in enumerate((heads[2 * hp], heads[2 * hp + 1])):
                    cols = w_attn[:, base + h * D : base + (h + 1) * D] * scl
                    for ct in range(8):
                        wqkv[blk, :, ct, s * 64 : (s + 1) * 64] = (
                            cols[ct * 128 : (ct + 1) * 128, :]
                        )
                    bq[s * 64 : (s + 1) * 64, blk] = (
                        b_attn[base + h * D : base + (h + 1) * D] * scl
                    )
        wproj_s = np.ascontiguousarray(
            np.concatenate([w_proj[h * D : (h + 1) * D, :] for h in heads], axis=0)
        )  # [256, 1024]
        xT = np.ascontiguousarray(x[b].T)  # [1024, 2048]
        in_maps.append(
            {
                "xT": xT.astype(BF16_NP),
                "wqkv": wqkv.astype(BF16_NP),
                "bqkv": bq,
                "wproj": wproj_s.astype(BF16_NP),
                "mask": mask.astype(BF16_NP),
                "ident": ident.astype(BF16_NP),
                "onesb": onesb,
                "onesv": onesv.astype(BF16_NP),
            }
        )
    return in_maps


_NC = None


def _get_nc():
    global _NC
    if _NC is None:
        _NC = build_nc()
    return _NC


def kernel(x, w_attn, b_attn, w_proj, b_proj):
    in_maps = host_shards(x, w_attn, b_attn, w_proj)
    nc = _get_nc()
    res = run_bass_kernel_spmd(nc, in_maps, core_ids=list(range(NCORES)))
    out = np.zeros((B, T, C), np.float64)
    for core in range(NCORES):
        out[core // 4] += res.results[core]["out"].astype(np.float64)
    out += np.asarray(b_proj, dtype=np.float64)[None, None, :]
    return out.astype(np.float32)
